# revision 34
# baseline (speedup 1.0000x reference)
"""Trainium2 Bass kernel for nn_DilatedAttention (B=2, L=4096, E=512, H=8, D=64,
dilation=2, window=256, causal, pre-norm transformer block with MLP).

Strategy
--------
* 8 cores, sequence-parallel: core c owns tokens [512c, 512c+512) of both
  batches, with a 256-token K/V halo before its range (zero-padded on core 0).
* Dilation-2 + causal + window couples only equal-parity positions; after
  de-interleaving by parity the mask is a causal sliding-window (window=128)
  attention over a length-2048 subsequence.  Parity de-interleave is a
  stride-2 slice on the free axis (feature-major layout).
* Both batches are interleaved instruction-by-instruction through every
  stage so each engine always has an independent stream to hide
  cross-engine dependency latency.
* Attention: per (head, parity) the scores for (query-block, key-slot) pairs
  land in one PSUM bank [128, 4, 128]; a single Exp per batch produces the
  probability tile, two strided affine_selects apply the sliding-window
  triangles for both batches at once, and the softmax denominator comes from
  a "valid key" ones-column appended to V.  Normalization is a single DVE
  divide per (head, parity, qblock).
* bf16 residual stream: LayerNorm statistics matmuls read the residual
  directly (no fp32->bf16 shadow copies); squares computed on DVE.
* LN decomposition: h = x*rstd + z with z = -mu*rstd; rstd/z broadcast
  across partitions via rank-1 PE matmuls; LN gains fold into weights on
  the host.
"""

import os
import sys
import types
import numpy as np
import ml_dtypes

import concourse.bass as bass
import concourse.mybir as mybir
import concourse.tile as tile
from concourse.bass_utils import run_bass_kernel_spmd
from concourse.masks import make_identity


def _install_ntff_hook_shim():
    """This image's antenv lacks axon_hooks; bass_utils imports it when
    BASS_TRACE is set.  Provide the ctypes-based NTFF hook (or a None hook)
    so tracing works — and never crashes — in any environment."""
    try:
        import antenv
    except ImportError:
        return
    try:
        from antenv.axon_hooks import get_axon_ntff_profile_hook  # noqa: F401
        return  # real module present
    except ImportError:
        pass
    import ctypes
    import contextlib

    hook = None
    so_path = "/opt/axon/libaxon_pjrt.so"
    if os.path.exists(so_path):
        try:
            lib = ctypes.CDLL(so_path)
            if hasattr(lib, "axon_start_nrt_profile"):
                lib.axon_start_nrt_profile.argtypes = [
                    ctypes.POINTER(ctypes.c_int64), ctypes.c_size_t]
                lib.axon_start_nrt_profile.restype = ctypes.c_int64
                lib.axon_stop_nrt_profile.argtypes = [ctypes.c_char_p]
                lib.axon_stop_nrt_profile.restype = ctypes.c_int64

                @contextlib.contextmanager
                def _hook(output_dir, device_ids):
                    import jax
                    jax.devices()
                    if device_ids:
                        ids = (ctypes.c_int64 * len(device_ids))(*device_ids)
                        rc = lib.axon_start_nrt_profile(ids, len(device_ids))
                    else:
                        rc = lib.axon_start_nrt_profile(None, 0)
                    if rc != 0:
                        raise RuntimeError(f"axon_start_nrt_profile rc={rc}")
                    try:
                        yield
                    finally:
                        lib.axon_stop_nrt_profile(str(output_dir).encode())

                hook = _hook
        except OSError:
            hook = None

    mod = types.ModuleType("antenv.axon_hooks")
    mod.get_axon_ntff_profile_hook = lambda: hook
    mod.set_axon_ntff_profile_hook = lambda h: None
    sys.modules["antenv.axon_hooks"] = mod
    antenv.axon_hooks = mod


_install_ntff_hook_shim()

F32 = mybir.dt.float32
BF16 = mybir.dt.bfloat16
AF = mybir.ActivationFunctionType
ALU = mybir.AluOpType

# problem constants
B, L, E, H, D = 2, 4096, 512, 8, 64
HID = 2048
EPS = 1e-5
WIN, DIL = 256, 2
N_CORES = 8
S = L // N_CORES          # tokens per core per batch (512)
HALO = WIN                # kv halo tokens (256)
T_EXT = S + HALO          # 768
EC = E // 128             # 4 feature chunks
HC = HID // 128           # 16 hidden chunks
NQ = S // 2               # queries per parity (256)
KB = (NQ + 128) // 128    # key blocks per parity (3)
QB = NQ // 128            # query blocks per parity (2)


def _legalize_waits(m, max_waits=1):
    """The walrus build here accepts only one sync-wait command per lowered
    instruction; hoist extras onto same-engine NoOps placed just before."""
    for fn in m.functions:
        for blk in fn.blocks:
            new_list = []
            for ins in blk.instructions:
                si = ins.sync_info
                if si is not None and si.on_wait is not None and len(si.on_wait) > max_waits:
                    waits = list(si.on_wait)
                    extra, keep = waits[:-max_waits], waits[-max_waits:]
                    k = 0
                    while extra:
                        chunk, extra = extra[:max_waits], extra[max_waits:]
                        nop = mybir.InstNoOp(name=f"{ins.name}-wsplit{k}", ins=[], outs=[])
                        nop.engine = ins.engine
                        nop.sync_info = mybir.SyncInfo(on_wait=chunk, on_update=[])
                        new_list.append(nop)
                        k += 1
                    si.on_wait = keep
                new_list.append(ins)
            blk.instructions = new_list


def build_program(has_qk_bias: bool, has_v_bias: bool, has_out_bias: bool, has_b2: bool):
    nc = bass.Bass("TRN2", target_bir_lowering=False, debug=False)

    # ---- DRAM I/O ----
    xT = nc.dram_tensor("xT", [B, E, T_EXT], BF16, kind="ExternalInput").ap()
    wqkv = nc.dram_tensor("wqkv", [E, 3 * E], BF16, kind="ExternalInput").ap()
    wout = nc.dram_tensor("wout", [E, E], BF16, kind="ExternalInput").ap()
    w1 = nc.dram_tensor("w1", [E, HID], BF16, kind="ExternalInput").ap()
    w2 = nc.dram_tensor("w2", [HID, E], BF16, kind="ExternalInput").ap()
    vones_in = nc.dram_tensor("vones", [KB * 128], BF16, kind="ExternalInput").ap()
    vmlp_in = nc.dram_tensor("vmlp", [HID], F32, kind="ExternalInput").ap()
    if has_qk_bias:
        vqk_in = nc.dram_tensor("vqk", [2 * E], F32, kind="ExternalInput").ap()
    if has_v_bias:
        vvb_in = nc.dram_tensor("vvb", [E], F32, kind="ExternalInput").ap()
    if has_out_bias:
        outb_in = nc.dram_tensor("outb", [E], F32, kind="ExternalInput").ap()
    if has_b2:
        b2_in = nc.dram_tensor("b2v", [E], F32, kind="ExternalInput").ap()
    yT = nc.dram_tensor("yT", [B, E, S], F32, kind="ExternalOutput").ap()

    with tile.TileContext(nc) as tc:
        ctxstack = []

        def pool(name, bufs, space="SBUF"):
            p = tc.tile_pool(name=name, bufs=bufs, space=space)
            ctxstack.append(p)
            return p.__enter__()

        wpool = pool("wpool", 1)
        xpool = pool("xpool", 2)
        bigpool = pool("bigpool", 2)   # x1 (stages A-B), then h2T (stage F)
        qkpool = pool("qkpool", 2)
        vpool = pool("vpool", 1)
        ptpool = pool("ptpool", 8)
        ospool = pool("ospool", 1)
        otpool = pool("otpool", 2)
        x2pool = pool("x2pool", 2)
        ypool = pool("ypool", 2)
        stpool = pool("stpool", 2)
        sqpool = pool("sqpool", 2)
        rpool = pool("rpool", 8)

        pstat = pool("pstat", 2, space="PSUM")
        pmain = pool("pmain", 3, space="PSUM")
        patt = pool("patt", 3, space="PSUM")

        # ---- x for both batches first (critical path), then weights.
        # Spread the startup loads over three DGE queues so both batches'
        # x and the qkv weights land in parallel. ----
        xts = []
        for b in range(B):
            xt = xpool.tile([128, EC, T_EXT], BF16, tag="xt", name=f"xt{b}")
            q = nc.sync if b == 0 else nc.gpsimd
            q.dma_start(xt, xT[b].rearrange("(c p) t -> p c t", p=128))
            xts.append(xt)
        wqkv_sb = wpool.tile([128, EC, 3 * E], BF16)
        nc.scalar.dma_start(wqkv_sb, wqkv.rearrange("(c p) f -> p c f", p=128))
        # small constants on the sync queue
        vmlp_sb = wpool.tile([128, HC], F32)
        nc.sync.dma_start(vmlp_sb, vmlp_in.rearrange("(s p) -> p s", p=128))
        vones_sb = wpool.tile([128, KB], BF16)
        nc.sync.dma_start(vones_sb, vones_in.rearrange("(k p) -> p k", p=128))
        if has_qk_bias:
            vqk_sb = wpool.tile([128, 8], F32)
            nc.sync.dma_start(vqk_sb, vqk_in.rearrange("(s p) -> p s", p=128))
        if has_v_bias:
            vvb_sb = wpool.tile([128, E], F32)
            nc.sync.dma_start(vvb_sb, vvb_in[None, :].to_broadcast([128, E]))
        if has_out_bias:
            outb_sb = wpool.tile([128, EC], F32)
            nc.sync.dma_start(outb_sb, outb_in.rearrange("(s p) -> p s", p=128))
        if has_b2:
            b2_sb = wpool.tile([128, EC], F32)
            nc.sync.dma_start(b2_sb, b2_in.rearrange("(s p) -> p s", p=128))
        # later-needed weights on the second (Activation) HWDGE queue
        wout_sb = wpool.tile([128, EC, E], BF16)
        nc.scalar.dma_start(wout_sb, wout.rearrange("(c p) f -> p c f", p=128))
        w1_sb = wpool.tile([128, EC, HID], BF16)
        nc.scalar.dma_start(w1_sb, w1.rearrange("(c p) f -> p c f", p=128))
        w2_sb = wpool.tile([128, HC, E], BF16)
        nc.scalar.dma_start(w2_sb, w2.rearrange("(c p) f -> p c f", p=128))

        ident = wpool.tile([128, 128], BF16)
        make_identity(nc, ident)
        ones_mat = wpool.tile([128, 128], BF16)
        nc.vector.memset(ones_mat, 1.0)
        eps_t = wpool.tile([128, 1], F32)
        nc.vector.memset(eps_t, EPS)

        def ln_rows(b):
            """Per-batch LN statistic slab tiles, replicated across all 128
            partitions so every chain op runs full-width (a [1,N] DVE op is
            ~6x slower than a [128,N] one).  Reused for LN1 then LN2."""
            return (stpool.tile([128, T_EXT], F32, tag="mu", name=f"mu{b}"),
                    stpool.tile([128, T_EXT], F32, tag="sttmp", name=f"sttmp{b}"),
                    stpool.tile([128, T_EXT], BF16, tag="strbf", name=f"strbf{b}"),
                    stpool.tile([128, T_EXT], BF16, tag="stz", name=f"stz{b}"))

        def ln_chunk(xt_ap, rows, t0, t1):
            """Emit the LN statistic chain for token range [t0, t1): per-token
            mean/var over the feature (partition x chunk) axis — replicated to
            all partitions via a ones-matrix matmul — then rstd =
            1/sqrt(var+eps) (bf16) and z = -mu*rstd."""
            mu_full, tmp_full, rstd_bf, zrow = rows
            mu_neg, tmp = mu_full[:, t0:t1], tmp_full[:, t0:t1]
            ps_s_full = pstat.tile([128, 384], F32, tag="pstat", name="ps_s_full")
            ps_s = ps_s_full[:, : t1 - t0]
            for c in range(EC):
                nc.tensor.matmul(ps_s, lhsT=ones_mat, rhs=xt_ap[:, c, t0:t1],
                                 start=(c == 0), stop=(c == EC - 1))
            nc.scalar.mul(mu_neg, ps_s, -1.0 / E)
            ps_q_full = pstat.tile([128, 384], F32, tag="pstat", name="ps_q_full")
            ps_q = ps_q_full[:, : t1 - t0]
            for c in range(EC):
                xsq_full = sqpool.tile([128, 512], BF16, tag="xsq", name="xsq")
                xsq = xsq_full[:, : t1 - t0]
                nc.vector.tensor_tensor(xsq, xt_ap[:, c, t0:t1],
                                        xt_ap[:, c, t0:t1], ALU.mult)
                nc.tensor.matmul(ps_q, lhsT=ones_mat, rhs=xsq,
                                 start=(c == 0), stop=(c == EC - 1))
            nc.scalar.mul(tmp, ps_q, 1.0 / E)
            # var = E[x^2] - mu^2 ; rstd = 1/sqrt(var+eps) ; z = -mu*rstd
            musq_full = sqpool.tile([128, 384], F32, tag="musq", name="musq")
            musq = musq_full[:, : t1 - t0]
            nc.scalar.square(musq, mu_neg)
            nc.vector.tensor_tensor(tmp, tmp, musq, ALU.subtract)
            nc.scalar.activation(tmp, tmp, AF.Sqrt, bias=eps_t)
            with nc.allow_low_precision(reason="rstd is consumed in bf16 anyway"):
                nc.vector.reciprocal(rstd_bf[:, t0:t1], tmp)
            nc.vector.tensor_tensor(zrow[:, t0:t1], mu_neg, rstd_bf[:, t0:t1],
                                    ALU.mult)

        # ---- stage A: LN1 stats + x1 = x*rstd + z (both batches).
        # The query/second-half token range (tt=1) is computed first so
        # stage B can start while the halo half's statistics are still in
        # flight. ----
        x1s = [bigpool.tile([128, EC, T_EXT], BF16, tag="big", name=f"x1_{b}")
               for b in range(B)]
        ln1_rows = [ln_rows(b) for b in range(B)]
        for tt in (1, 0):
            t0, t1 = tt * 384, (tt + 1) * 384
            for b in range(B):
                ln_chunk(xts[b], ln1_rows[b], t0, t1)
            for b in range(B):
                rstd_bf, zrow = ln1_rows[b][2], ln1_rows[b][3]
                x1 = x1s[b]
                for c in range(EC):
                    eng = nc.vector if c < 2 else nc.gpsimd
                    eng.tensor_tensor(x1[:, c, t0:t1], xts[b][:, c, t0:t1],
                                      rstd_bf[:, t0:t1], ALU.mult)
                    eng.tensor_tensor(x1[:, c, t0:t1], x1[:, c, t0:t1],
                                      zrow[:, t0:t1], ALU.add)

        # ---- stage B: QKV, batches interleaved per feature slice ----
        qkTs = [qkpool.tile([128, 8, T_EXT], BF16, tag="qkT", name=f"qkT{b}")
                for b in range(B)]
        for fs in range(8):
            # queries only need the core's own tokens; keys need the halo.
            # tt=1 ranges first: their x1 is ready earliest.
            spans = [(HALO, T_EXT)] if fs < 4 else [(384, T_EXT), (0, 384)]
            for t0, t1 in spans:
                for b in range(B):
                    ps_full = pmain.tile([128, 512], F32, tag="pmain", name="ps_full")
                    ps = ps_full[:, : t1 - t0]
                    for c in range(EC):
                        nc.tensor.matmul(ps, lhsT=wqkv_sb[:, c, fs * 128:(fs + 1) * 128],
                                         rhs=x1s[b][:, c, t0:t1],
                                         start=(c == 0), stop=(c == EC - 1))
                    qkT = qkTs[b]
                    if has_qk_bias:
                        if fs >= 4:
                            nc.vector.tensor_scalar(qkT[:, fs, t0:t1], ps,
                                                    vqk_sb[:, fs:fs + 1], None, ALU.add)
                        else:
                            nc.scalar.activation(qkT[:, fs, t0:t1], ps, AF.Identity,
                                                 bias=vqk_sb[:, fs:fs + 1])
                    else:
                        nc.scalar.copy(qkT[:, fs, t0:t1], ps)

        # V token-major, parity-separated, with the valid-key column appended
        vplus = [[vpool.tile([128, KB, H, D + 1], BF16, tag=f"vplus{b}{p}",
                             name=f"vplus{b}{p}") for p in range(2)]
                 for b in range(B)]
        for b in range(B):
            for par in range(2):
                nc.gpsimd.tensor_copy(vplus[b][par][:, :, :, D],
                                      vones_sb[:, :, None].to_broadcast([128, KB, H]))
        x1_pars = [x1s[b].rearrange("p c (t two) -> p c two t", two=2) for b in range(B)]
        for par in range(2):
            for kb in reversed(range(KB)):
                for b in range(B):
                    ps = pmain.tile([128, E], F32, tag="pmain")
                    for c in range(EC):
                        nc.tensor.matmul(
                            ps, lhsT=x1_pars[b][:, c, par, kb * 128:(kb + 1) * 128],
                            rhs=wqkv_sb[:, c, 2 * E:3 * E],
                            start=(c == 0), stop=(c == EC - 1))
                    pv = ps.rearrange("p (h d) -> p h d", h=H)
                    if has_v_bias:
                        nc.vector.tensor_tensor(vplus[b][par][:, kb, :, 0:D], pv,
                                                vvb_sb.rearrange("p (h d) -> p h d", h=H),
                                                ALU.add)
                    else:
                        nc.scalar.copy(vplus[b][par][:, kb, :, 0:D], pv)

        # ---- stage C: attention ----
        # scores per (par, h): PSUM [128 keys, (qb,slot), 128 queries], where
        # slot 0 -> key block qb (keep k>=q), slot 1 -> key block qb+1
        # (keep q>=k).  One Exp per batch; masks span both batches via the
        # joint pt tile [128, B, 4, 128].
        qkT_pars = [qkTs[b].rearrange("p s (t two) -> p s two t", two=2)
                    for b in range(B)]
        oslabs = [[ospool.tile([128, QB, E], BF16, tag=f"oslab{b}{p}",
                               name=f"oslab{b}{p}") for p in range(2)]
                  for b in range(B)]
        for par in range(2):
            for h in range(H):
                rb = (h % 2) * 64
                sl = h // 2
                pt = ptpool.tile([128, B, 2 * QB, 128], BF16, tag="pt")
                for b in range(B):
                    ps_sc = pmain.tile([128, 2 * QB, 128], F32, tag="pmain",
                                       name="ps_sc")
                    for qb in range(QB):
                        for slot in range(2):
                            kb = qb + slot
                            nc.tensor.matmul(
                                ps_sc[:, 2 * qb + slot, :],
                                lhsT=qkT_pars[b][rb:rb + 64, 4 + sl, par,
                                                 kb * 128:(kb + 1) * 128],
                                rhs=qkT_pars[b][rb:rb + 64, sl, par,
                                                128 + qb * 128:256 + qb * 128],
                                start=True, stop=True)
                    nc.scalar.activation(pt[:, b], ps_sc, AF.Exp)
                # sliding-window masks, both batches at once:
                # lo slots: keep k - q >= 0 ; hi slots: keep q - k >= 0
                nc.gpsimd.affine_select(
                    out=pt[:, :, 0::2, :], in_=pt[:, :, 0::2, :],
                    compare_op=ALU.is_ge, fill=0.0,
                    base=0, channel_multiplier=1,
                    pattern=[[0, B], [0, QB], [-1, 128]])
                nc.gpsimd.affine_select(
                    out=pt[:, :, 1::2, :], in_=pt[:, :, 1::2, :],
                    compare_op=ALU.is_ge, fill=0.0,
                    base=0, channel_multiplier=-1,
                    pattern=[[0, B], [0, QB], [1, 128]])
                for b in range(B):
                    ps_o = patt.tile([128, QB, D + 1], F32, tag="patt")
                    for qb in range(QB):
                        for slot in range(2):
                            nc.tensor.matmul(ps_o[:, qb, :],
                                             lhsT=pt[:, b, 2 * qb + slot, :],
                                             rhs=vplus[b][par][:, qb + slot, h, :],
                                             start=(slot == 0), stop=(slot == 1))
                    rin = rpool.tile([128, QB], F32, tag="rin", name="rin")
                    nc.vector.reciprocal(rin, ps_o[:, :, D])
                    for qb in range(QB):
                        nc.vector.tensor_scalar(
                            oslabs[b][par][:, qb, h * D:(h + 1) * D],
                            ps_o[:, qb, 0:D], rin[:, qb:qb + 1], None, ALU.mult)

        # transpose O back to feature-major, re-interleaving parities
        otTs = [otpool.tile([128, EC, S], BF16, tag="otT", name=f"otT{b}")
                for b in range(B)]
        otT_pars = [otTs[b].rearrange("p c (t two) -> p c two t", two=2)
                    for b in range(B)]
        for par in range(2):
            for qb in range(QB):
                for fc in range(EC):
                    for b in range(B):
                        ps_t = patt.tile([128, 128], BF16, tag="patt", name="ps_t")
                        nc.tensor.transpose(
                            ps_t, oslabs[b][par][:, qb, fc * 128:(fc + 1) * 128], ident)
                        if b == 0:
                            nc.vector.tensor_copy(
                                otT_pars[b][:, fc, par, qb * 128:(qb + 1) * 128], ps_t)
                        else:
                            nc.scalar.copy(
                                otT_pars[b][:, fc, par, qb * 128:(qb + 1) * 128], ps_t)

        # ---- stage D: out-proj + residual ----
        x2Ts = [x2pool.tile([128, EC, S], BF16, tag="x2T", name=f"x2T{b}")
                for b in range(B)]
        for es in range(EC):
            for b in range(B):
                ps = pmain.tile([128, S], F32, tag="pmain")
                for c in range(EC):
                    nc.tensor.matmul(ps, lhsT=wout_sb[:, c, es * 128:(es + 1) * 128],
                                     rhs=otTs[b][:, c, :], start=(c == 0), stop=(c == EC - 1))
                if has_out_bias:
                    nc.vector.tensor_scalar(ps, ps, outb_sb[:, es:es + 1], None, ALU.add)
                nc.vector.tensor_tensor(x2Ts[b][:, es, :], ps,
                                        xts[b][:, es, HALO:T_EXT], ALU.add)

        # ---- stage E: LN2 ----
        ln2_rows = [ln_rows(b) for b in range(B)]
        for tt in range(2):
            for b in range(B):
                ln_chunk(x2Ts[b], ln2_rows[b], tt * 256, (tt + 1) * 256)
        x21s = []
        for b in range(B):
            x2T = x2Ts[b]
            rstd2_bf, z2row = ln2_rows[b][2][:, :S], ln2_rows[b][3][:, :S]
            x21 = x2pool.tile([128, EC, S], BF16, tag="x21", name=f"x21_{b}")
            for c in range(EC):
                eng = nc.vector if c < 2 else nc.gpsimd
                eng.tensor_tensor(x21[:, c, :], x2T[:, c, :], rstd2_bf, ALU.mult)
                eng.tensor_tensor(x21[:, c, :], x21[:, c, :], z2row, ALU.add)
            x21s.append(x21)

        # ---- stage F: MLP, batches interleaved ----
        h2Ts = [bigpool.tile([128, HC, S], BF16, tag="big", name=f"h2T{b}")
                for b in range(B)]
        for hs in range(HC):
            for b in range(B):
                ps = pmain.tile([128, S], F32, tag="pmain")
                for c in range(EC):
                    nc.tensor.matmul(ps, lhsT=w1_sb[:, c, hs * 128:(hs + 1) * 128],
                                     rhs=x21s[b][:, c, :], start=(c == 0), stop=(c == EC - 1))
                nc.scalar.activation(h2Ts[b][:, hs, :], ps, AF.Gelu,
                                     bias=vmlp_sb[:, hs:hs + 1])
        for es in range(EC):
            for b in range(B):
                ps = pmain.tile([128, S], F32, tag="pmain")
                for hc in range(HC):
                    nc.tensor.matmul(ps, lhsT=w2_sb[:, hc, es * 128:(es + 1) * 128],
                                     rhs=h2Ts[b][:, hc, :], start=(hc == 0), stop=(hc == HC - 1))
                if has_b2:
                    nc.vector.tensor_scalar(ps, ps, b2_sb[:, es:es + 1], None, ALU.add)
                yt = ypool.tile([128, S], F32, tag="yt", name="yt")
                nc.vector.tensor_tensor(yt, ps, x2Ts[b][:, es, :], ALU.add)
                nc.sync.dma_start(yT[b, es * 128:(es + 1) * 128, :], yt)

        for p in reversed(ctxstack):
            p.__exit__(None, None, None)

    return nc


_cached = {}


def _get_program(key):
    if key not in _cached:
        nc = build_program(*key)
        _legalize_waits(nc.m)
        _cached[key] = nc
    return _cached[key]


def _prepare_core_inputs(inputs):
    """Host-side folding + sharding. Returns (flags_key, in_maps list)."""
    x = np.asarray(inputs["x"], np.float32)
    ln1_g = np.asarray(inputs["ln1_g"], np.float32)
    ln1_b = np.asarray(inputs["ln1_b"], np.float32)
    qkv_w = np.asarray(inputs["qkv_w"], np.float32)
    qkv_b = np.asarray(inputs["qkv_b"], np.float32)
    out_w = np.asarray(inputs["out_w"], np.float32)
    out_b = np.asarray(inputs["out_b"], np.float32)
    ln2_g = np.asarray(inputs["ln2_g"], np.float32)
    ln2_b = np.asarray(inputs["ln2_b"], np.float32)
    w1 = np.asarray(inputs["w1"], np.float32)
    b1 = np.asarray(inputs["b1"], np.float32)
    w2 = np.asarray(inputs["w2"], np.float32)
    b2 = np.asarray(inputs["b2"], np.float32)

    # fold LN1 gain into qkv_w; fold attention 1/sqrt(D) into the Q part
    qscale = 1.0 / np.sqrt(D)
    wqkv_eff = ln1_g[:, None] * qkv_w
    vqkv = ln1_b @ qkv_w + qkv_b          # [3E]
    wqkv_eff[:, :E] *= qscale
    vqkv = vqkv.copy()
    vqkv[:E] *= qscale
    # fold LN2 gain into w1
    w1_eff = ln2_g[:, None] * w1
    vmlp = ln2_b @ w1 + b1                # [HID]

    has_qk_bias = bool(np.any(vqkv[: 2 * E] != 0.0))
    has_v_bias = bool(np.any(vqkv[2 * E:] != 0.0))
    has_out_bias = bool(np.any(out_b != 0.0))
    has_b2 = bool(np.any(b2 != 0.0))
    key = (has_qk_bias, has_v_bias, has_out_bias, has_b2)

    wqkv_bf = wqkv_eff.astype(ml_dtypes.bfloat16)
    wout_bf = out_w.astype(ml_dtypes.bfloat16)
    w1_bf = w1_eff.astype(ml_dtypes.bfloat16)
    w2_bf = w2.astype(ml_dtypes.bfloat16)

    # x transposed per batch with halo: [B, E, T_EXT], bf16 residual stream
    xT_full = np.ascontiguousarray(x.transpose(0, 2, 1))  # [B, E, L]
    in_maps = []
    for c in range(N_CORES):
        s = c * S
        xTe = np.zeros((B, E, T_EXT), np.float32)
        lo = s - HALO
        src_lo = max(lo, 0)
        xTe[:, :, src_lo - lo:] = xT_full[:, :, src_lo:s + S]
        vones = np.ones(KB * 128, np.float32)
        if c == 0:
            vones[:128] = 0.0
        im = {
            "xT": xTe.astype(ml_dtypes.bfloat16),
            "wqkv": wqkv_bf,
            "wout": wout_bf,
            "w1": w1_bf,
            "w2": w2_bf,
            "vones": vones.astype(ml_dtypes.bfloat16),
            "vmlp": vmlp.astype(np.float32),
        }
        if has_qk_bias:
            im["vqk"] = vqkv[: 2 * E].astype(np.float32)
        if has_v_bias:
            im["vvb"] = vqkv[2 * E:].astype(np.float32)
        if has_out_bias:
            im["outb"] = out_b.astype(np.float32)
        if has_b2:
            im["b2v"] = b2.astype(np.float32)
        in_maps.append(im)
    return key, in_maps


_last_results = None


def kernel(**inputs) -> np.ndarray:
    global _last_results
    key, in_maps = _prepare_core_inputs(inputs)
    nc = _get_program(key)
    res = run_bass_kernel_spmd(nc, in_maps, core_ids=list(range(N_CORES)))
    _last_results = res
    out = np.empty((B, L, E), np.float32)
    for c in range(N_CORES):
        yT = res.results[c]["yT"]          # [B, E, S]
        out[:, c * S:(c + 1) * S, :] = yT.transpose(0, 2, 1)
    return out


# revision 35
# speedup vs baseline: 1.0052x; 1.0052x over previous
"""Trainium2 Bass kernel for nn_DilatedAttention (B=2, L=4096, E=512, H=8, D=64,
dilation=2, window=256, causal, pre-norm transformer block with MLP).

Strategy
--------
* 8 cores, sequence-parallel: core c owns tokens [512c, 512c+512) of both
  batches, with a 256-token K/V halo before its range (zero-padded on core 0).
* Dilation-2 + causal + window couples only equal-parity positions; after
  de-interleaving by parity the mask is a causal sliding-window (window=128)
  attention over a length-2048 subsequence.  Parity de-interleave is a
  stride-2 slice on the free axis (feature-major layout).
* Both batches are interleaved instruction-by-instruction through every
  stage so each engine always has an independent stream to hide
  cross-engine dependency latency.
* Attention: per (head, parity) the scores for (query-block, key-slot) pairs
  land in one PSUM bank [128, 4, 128]; a single Exp per batch produces the
  probability tile, two strided affine_selects apply the sliding-window
  triangles for both batches at once, and the softmax denominator comes from
  a "valid key" ones-column appended to V.  Normalization is a single DVE
  divide per (head, parity, qblock).
* bf16 residual stream: LayerNorm statistics matmuls read the residual
  directly (no fp32->bf16 shadow copies); squares computed on DVE.
* LN decomposition: h = x*rstd + z with z = -mu*rstd; rstd/z broadcast
  across partitions via rank-1 PE matmuls; LN gains fold into weights on
  the host.
"""

import os
import sys
import types
import numpy as np
import ml_dtypes

import concourse.bass as bass
import concourse.mybir as mybir
import concourse.tile as tile
from concourse.bass_utils import run_bass_kernel_spmd
from concourse.masks import make_identity


def _install_ntff_hook_shim():
    """This image's antenv lacks axon_hooks; bass_utils imports it when
    BASS_TRACE is set.  Provide the ctypes-based NTFF hook (or a None hook)
    so tracing works — and never crashes — in any environment."""
    try:
        import antenv
    except ImportError:
        return
    try:
        from antenv.axon_hooks import get_axon_ntff_profile_hook  # noqa: F401
        return  # real module present
    except ImportError:
        pass
    import ctypes
    import contextlib

    hook = None
    so_path = "/opt/axon/libaxon_pjrt.so"
    if os.path.exists(so_path):
        try:
            lib = ctypes.CDLL(so_path)
            if hasattr(lib, "axon_start_nrt_profile"):
                lib.axon_start_nrt_profile.argtypes = [
                    ctypes.POINTER(ctypes.c_int64), ctypes.c_size_t]
                lib.axon_start_nrt_profile.restype = ctypes.c_int64
                lib.axon_stop_nrt_profile.argtypes = [ctypes.c_char_p]
                lib.axon_stop_nrt_profile.restype = ctypes.c_int64

                @contextlib.contextmanager
                def _hook(output_dir, device_ids):
                    import jax
                    jax.devices()
                    if device_ids:
                        ids = (ctypes.c_int64 * len(device_ids))(*device_ids)
                        rc = lib.axon_start_nrt_profile(ids, len(device_ids))
                    else:
                        rc = lib.axon_start_nrt_profile(None, 0)
                    if rc != 0:
                        raise RuntimeError(f"axon_start_nrt_profile rc={rc}")
                    try:
                        yield
                    finally:
                        lib.axon_stop_nrt_profile(str(output_dir).encode())

                hook = _hook
        except OSError:
            hook = None

    mod = types.ModuleType("antenv.axon_hooks")
    mod.get_axon_ntff_profile_hook = lambda: hook
    mod.set_axon_ntff_profile_hook = lambda h: None
    sys.modules["antenv.axon_hooks"] = mod
    antenv.axon_hooks = mod


_install_ntff_hook_shim()

F32 = mybir.dt.float32
BF16 = mybir.dt.bfloat16
AF = mybir.ActivationFunctionType
ALU = mybir.AluOpType

# problem constants
B, L, E, H, D = 2, 4096, 512, 8, 64
HID = 2048
EPS = 1e-5
WIN, DIL = 256, 2
N_CORES = 8
S = L // N_CORES          # tokens per core per batch (512)
HALO = WIN                # kv halo tokens (256)
T_EXT = S + HALO          # 768
EC = E // 128             # 4 feature chunks
HC = HID // 128           # 16 hidden chunks
NQ = S // 2               # queries per parity (256)
KB = (NQ + 128) // 128    # key blocks per parity (3)
QB = NQ // 128            # query blocks per parity (2)


def _legalize_waits(m, max_waits=1):
    """The walrus build here accepts only one sync-wait command per lowered
    instruction; hoist extras onto same-engine NoOps placed just before."""
    for fn in m.functions:
        for blk in fn.blocks:
            new_list = []
            for ins in blk.instructions:
                si = ins.sync_info
                if si is not None and si.on_wait is not None and len(si.on_wait) > max_waits:
                    waits = list(si.on_wait)
                    extra, keep = waits[:-max_waits], waits[-max_waits:]
                    k = 0
                    while extra:
                        chunk, extra = extra[:max_waits], extra[max_waits:]
                        nop = mybir.InstNoOp(name=f"{ins.name}-wsplit{k}", ins=[], outs=[])
                        nop.engine = ins.engine
                        nop.sync_info = mybir.SyncInfo(on_wait=chunk, on_update=[])
                        new_list.append(nop)
                        k += 1
                    si.on_wait = keep
                new_list.append(ins)
            blk.instructions = new_list


def build_program(has_qk_bias: bool, has_v_bias: bool, has_out_bias: bool, has_b2: bool):
    nc = bass.Bass("TRN2", target_bir_lowering=False, debug=False)

    # ---- DRAM I/O ----
    xT = nc.dram_tensor("xT", [B, E, T_EXT], BF16, kind="ExternalInput").ap()
    wqkv = nc.dram_tensor("wqkv", [E, 3 * E], BF16, kind="ExternalInput").ap()
    wout = nc.dram_tensor("wout", [E, E], BF16, kind="ExternalInput").ap()
    w1 = nc.dram_tensor("w1", [E, HID], BF16, kind="ExternalInput").ap()
    w2 = nc.dram_tensor("w2", [HID, E], BF16, kind="ExternalInput").ap()
    vones_in = nc.dram_tensor("vones", [KB * 128], BF16, kind="ExternalInput").ap()
    vmlp_in = nc.dram_tensor("vmlp", [HID], F32, kind="ExternalInput").ap()
    if has_qk_bias:
        vqk_in = nc.dram_tensor("vqk", [2 * E], F32, kind="ExternalInput").ap()
    if has_v_bias:
        vvb_in = nc.dram_tensor("vvb", [E], F32, kind="ExternalInput").ap()
    if has_out_bias:
        outb_in = nc.dram_tensor("outb", [E], F32, kind="ExternalInput").ap()
    if has_b2:
        b2_in = nc.dram_tensor("b2v", [E], F32, kind="ExternalInput").ap()
    yT = nc.dram_tensor("yT", [B, E, S], F32, kind="ExternalOutput").ap()

    with tile.TileContext(nc) as tc:
        ctxstack = []

        def pool(name, bufs, space="SBUF"):
            p = tc.tile_pool(name=name, bufs=bufs, space=space)
            ctxstack.append(p)
            return p.__enter__()

        wpool = pool("wpool", 1)
        xpool = pool("xpool", 2)
        bigpool = pool("bigpool", 2)   # x1 (stages A-B), then h2T (stage F)
        qkpool = pool("qkpool", 2)
        vpool = pool("vpool", 1)
        ptpool = pool("ptpool", 6)
        ospool = pool("ospool", 1)
        otpool = pool("otpool", 2)
        x2pool = pool("x2pool", 2)
        ypool = pool("ypool", 2)
        stpool = pool("stpool", 2)
        sqpool = pool("sqpool", 2)
        rpool = pool("rpool", 4)

        pstat = pool("pstat", 2, space="PSUM")
        pmain = pool("pmain", 3, space="PSUM")
        patt = pool("patt", 3, space="PSUM")

        # ---- x for both batches first (critical path), then weights.
        # Spread the startup loads over three DGE queues so both batches'
        # x and the qkv weights land in parallel. ----
        xts = []
        for b in range(B):
            xt = xpool.tile([128, EC, T_EXT], BF16, tag="xt", name=f"xt{b}")
            q = nc.sync if b == 0 else nc.gpsimd
            q.dma_start(xt, xT[b].rearrange("(c p) t -> p c t", p=128))
            xts.append(xt)
        wqkv_sb = wpool.tile([128, EC, 3 * E], BF16)
        nc.scalar.dma_start(wqkv_sb, wqkv.rearrange("(c p) f -> p c f", p=128))
        # small constants on the sync queue
        vmlp_sb = wpool.tile([128, HC], F32)
        nc.sync.dma_start(vmlp_sb, vmlp_in.rearrange("(s p) -> p s", p=128))
        vones_sb = wpool.tile([128, KB], BF16)
        nc.sync.dma_start(vones_sb, vones_in.rearrange("(k p) -> p k", p=128))
        if has_qk_bias:
            vqk_sb = wpool.tile([128, 8], F32)
            nc.sync.dma_start(vqk_sb, vqk_in.rearrange("(s p) -> p s", p=128))
        if has_v_bias:
            vvb_sb = wpool.tile([128, E], F32)
            nc.sync.dma_start(vvb_sb, vvb_in[None, :].to_broadcast([128, E]))
        if has_out_bias:
            outb_sb = wpool.tile([128, EC], F32)
            nc.sync.dma_start(outb_sb, outb_in.rearrange("(s p) -> p s", p=128))
        if has_b2:
            b2_sb = wpool.tile([128, EC], F32)
            nc.sync.dma_start(b2_sb, b2_in.rearrange("(s p) -> p s", p=128))
        # later-needed weights on the second (Activation) HWDGE queue
        wout_sb = wpool.tile([128, EC, E], BF16)
        nc.scalar.dma_start(wout_sb, wout.rearrange("(c p) f -> p c f", p=128))
        w1_sb = wpool.tile([128, EC, HID], BF16)
        nc.scalar.dma_start(w1_sb, w1.rearrange("(c p) f -> p c f", p=128))
        w2_sb = wpool.tile([128, HC, E], BF16)
        nc.scalar.dma_start(w2_sb, w2.rearrange("(c p) f -> p c f", p=128))

        ident = wpool.tile([128, 128], BF16)
        make_identity(nc, ident)
        ones_mat = wpool.tile([128, 128], BF16)
        nc.vector.memset(ones_mat, 1.0)
        eps_t = wpool.tile([128, 1], F32)
        nc.vector.memset(eps_t, EPS)

        def ln_rows(b):
            """Per-batch LN statistic slab tiles, replicated across all 128
            partitions so every chain op runs full-width (a [1,N] DVE op is
            ~6x slower than a [128,N] one).  Reused for LN1 then LN2."""
            return (stpool.tile([128, T_EXT], F32, tag="mu", name=f"mu{b}"),
                    stpool.tile([128, T_EXT], F32, tag="sttmp", name=f"sttmp{b}"),
                    stpool.tile([128, T_EXT], BF16, tag="strbf", name=f"strbf{b}"),
                    stpool.tile([128, T_EXT], BF16, tag="stz", name=f"stz{b}"))

        def ln_chunk(xt_ap, rows, t0, t1):
            """Emit the LN statistic chain for token range [t0, t1): per-token
            mean/var over the feature (partition x chunk) axis — replicated to
            all partitions via a ones-matrix matmul — then rstd =
            1/sqrt(var+eps) (bf16) and z = -mu*rstd."""
            mu_full, tmp_full, rstd_bf, zrow = rows
            mu_neg, tmp = mu_full[:, t0:t1], tmp_full[:, t0:t1]
            ps_s_full = pstat.tile([128, 384], F32, tag="pstat", name="ps_s_full")
            ps_s = ps_s_full[:, : t1 - t0]
            for c in range(EC):
                nc.tensor.matmul(ps_s, lhsT=ones_mat, rhs=xt_ap[:, c, t0:t1],
                                 start=(c == 0), stop=(c == EC - 1))
            nc.scalar.mul(mu_neg, ps_s, -1.0 / E)
            ps_q_full = pstat.tile([128, 384], F32, tag="pstat", name="ps_q_full")
            ps_q = ps_q_full[:, : t1 - t0]
            for c in range(EC):
                xsq_full = sqpool.tile([128, 512], BF16, tag="xsq", name="xsq")
                xsq = xsq_full[:, : t1 - t0]
                nc.vector.tensor_tensor(xsq, xt_ap[:, c, t0:t1],
                                        xt_ap[:, c, t0:t1], ALU.mult)
                nc.tensor.matmul(ps_q, lhsT=ones_mat, rhs=xsq,
                                 start=(c == 0), stop=(c == EC - 1))
            nc.scalar.mul(tmp, ps_q, 1.0 / E)
            # var = E[x^2] - mu^2 ; rstd = 1/sqrt(var+eps) ; z = -mu*rstd
            musq_full = sqpool.tile([128, 384], F32, tag="musq", name="musq")
            musq = musq_full[:, : t1 - t0]
            nc.scalar.square(musq, mu_neg)
            nc.vector.tensor_tensor(tmp, tmp, musq, ALU.subtract)
            nc.scalar.activation(tmp, tmp, AF.Sqrt, bias=eps_t)
            with nc.allow_low_precision(reason="rstd is consumed in bf16 anyway"):
                nc.vector.reciprocal(rstd_bf[:, t0:t1], tmp)
            nc.vector.tensor_tensor(zrow[:, t0:t1], mu_neg, rstd_bf[:, t0:t1],
                                    ALU.mult)

        # ---- stage A: LN1 stats + x1 = x*rstd + z (both batches).
        # The query/second-half token range (tt=1) is computed first so
        # stage B can start while the halo half's statistics are still in
        # flight. ----
        x1s = [bigpool.tile([128, EC, T_EXT], BF16, tag="big", name=f"x1_{b}")
               for b in range(B)]
        ln1_rows = [ln_rows(b) for b in range(B)]
        for tt in (1, 0):
            t0, t1 = tt * 384, (tt + 1) * 384
            for b in range(B):
                ln_chunk(xts[b], ln1_rows[b], t0, t1)
            for b in range(B):
                rstd_bf, zrow = ln1_rows[b][2], ln1_rows[b][3]
                x1 = x1s[b]
                for c in range(EC):
                    eng = nc.vector if c < 2 else nc.gpsimd
                    eng.tensor_tensor(x1[:, c, t0:t1], xts[b][:, c, t0:t1],
                                      rstd_bf[:, t0:t1], ALU.mult)
                    eng.tensor_tensor(x1[:, c, t0:t1], x1[:, c, t0:t1],
                                      zrow[:, t0:t1], ALU.add)

        # ---- stage B: QKV, batches interleaved per feature slice ----
        qkTs = [qkpool.tile([128, 8, T_EXT], BF16, tag="qkT", name=f"qkT{b}")
                for b in range(B)]
        for fs in range(8):
            # queries only need the core's own tokens; keys need the halo.
            # tt=1 ranges first: their x1 is ready earliest.
            spans = [(HALO, T_EXT)] if fs < 4 else [(384, T_EXT), (0, 384)]
            for t0, t1 in spans:
                for b in range(B):
                    ps_full = pmain.tile([128, 512], F32, tag="pmain", name="ps_full")
                    ps = ps_full[:, : t1 - t0]
                    for c in range(EC):
                        nc.tensor.matmul(ps, lhsT=wqkv_sb[:, c, fs * 128:(fs + 1) * 128],
                                         rhs=x1s[b][:, c, t0:t1],
                                         start=(c == 0), stop=(c == EC - 1))
                    qkT = qkTs[b]
                    if has_qk_bias:
                        if fs >= 4:
                            nc.vector.tensor_scalar(qkT[:, fs, t0:t1], ps,
                                                    vqk_sb[:, fs:fs + 1], None, ALU.add)
                        else:
                            nc.scalar.activation(qkT[:, fs, t0:t1], ps, AF.Identity,
                                                 bias=vqk_sb[:, fs:fs + 1])
                    else:
                        nc.scalar.copy(qkT[:, fs, t0:t1], ps)

        # V token-major, parity-separated, with the valid-key column appended
        vplus = [[vpool.tile([128, KB, H, D + 1], BF16, tag=f"vplus{b}{p}",
                             name=f"vplus{b}{p}") for p in range(2)]
                 for b in range(B)]
        for b in range(B):
            for par in range(2):
                nc.gpsimd.tensor_copy(vplus[b][par][:, :, :, D],
                                      vones_sb[:, :, None].to_broadcast([128, KB, H]))
        x1_pars = [x1s[b].rearrange("p c (t two) -> p c two t", two=2) for b in range(B)]
        for par in range(2):
            for kb in reversed(range(KB)):
                for b in range(B):
                    ps = pmain.tile([128, E], F32, tag="pmain")
                    for c in range(EC):
                        nc.tensor.matmul(
                            ps, lhsT=x1_pars[b][:, c, par, kb * 128:(kb + 1) * 128],
                            rhs=wqkv_sb[:, c, 2 * E:3 * E],
                            start=(c == 0), stop=(c == EC - 1))
                    pv = ps.rearrange("p (h d) -> p h d", h=H)
                    if has_v_bias:
                        nc.vector.tensor_tensor(vplus[b][par][:, kb, :, 0:D], pv,
                                                vvb_sb.rearrange("p (h d) -> p h d", h=H),
                                                ALU.add)
                    else:
                        nc.scalar.copy(vplus[b][par][:, kb, :, 0:D], pv)

        # ---- stage C: attention ----
        # scores per (par, h): PSUM [128 keys, (qb,slot), 128 queries], where
        # slot 0 -> key block qb (keep k>=q), slot 1 -> key block qb+1
        # (keep q>=k).  One Exp per batch; masks span both batches via the
        # joint pt tile [128, B, 4, 128].
        qkT_pars = [qkTs[b].rearrange("p s (t two) -> p s two t", two=2)
                    for b in range(B)]
        oslabs = [[ospool.tile([128, QB, E], BF16, tag=f"oslab{b}{p}",
                               name=f"oslab{b}{p}") for p in range(2)]
                  for b in range(B)]
        for par in range(2):
            for h in range(H):
                rb = (h % 2) * 64
                sl = h // 2
                pt = ptpool.tile([128, B, 2 * QB, 128], BF16, tag="pt")
                for b in range(B):
                    ps_sc = pmain.tile([128, 2 * QB, 128], F32, tag="pmain",
                                       name="ps_sc")
                    for qb in range(QB):
                        for slot in range(2):
                            kb = qb + slot
                            nc.tensor.matmul(
                                ps_sc[:, 2 * qb + slot, :],
                                lhsT=qkT_pars[b][rb:rb + 64, 4 + sl, par,
                                                 kb * 128:(kb + 1) * 128],
                                rhs=qkT_pars[b][rb:rb + 64, sl, par,
                                                128 + qb * 128:256 + qb * 128],
                                start=True, stop=True)
                    nc.scalar.activation(pt[:, b], ps_sc, AF.Exp)
                # sliding-window masks, both batches at once:
                # lo slots: keep k - q >= 0 ; hi slots: keep q - k >= 0
                nc.gpsimd.affine_select(
                    out=pt[:, :, 0::2, :], in_=pt[:, :, 0::2, :],
                    compare_op=ALU.is_ge, fill=0.0,
                    base=0, channel_multiplier=1,
                    pattern=[[0, B], [0, QB], [-1, 128]])
                nc.gpsimd.affine_select(
                    out=pt[:, :, 1::2, :], in_=pt[:, :, 1::2, :],
                    compare_op=ALU.is_ge, fill=0.0,
                    base=0, channel_multiplier=-1,
                    pattern=[[0, B], [0, QB], [1, 128]])
                for b in range(B):
                    ps_o = patt.tile([128, QB, D + 1], F32, tag="patt")
                    for qb in range(QB):
                        for slot in range(2):
                            nc.tensor.matmul(ps_o[:, qb, :],
                                             lhsT=pt[:, b, 2 * qb + slot, :],
                                             rhs=vplus[b][par][:, qb + slot, h, :],
                                             start=(slot == 0), stop=(slot == 1))
                    rin = rpool.tile([128, QB], F32, tag="rin", name="rin")
                    nc.vector.reciprocal(rin, ps_o[:, :, D])
                    for qb in range(QB):
                        nc.vector.tensor_scalar(
                            oslabs[b][par][:, qb, h * D:(h + 1) * D],
                            ps_o[:, qb, 0:D], rin[:, qb:qb + 1], None, ALU.mult)

        # transpose O back to feature-major, re-interleaving parities
        otTs = [otpool.tile([128, EC, S], BF16, tag="otT", name=f"otT{b}")
                for b in range(B)]
        otT_pars = [otTs[b].rearrange("p c (t two) -> p c two t", two=2)
                    for b in range(B)]
        for par in range(2):
            for qb in range(QB):
                for fc in range(EC):
                    for b in range(B):
                        ps_t = patt.tile([128, 128], BF16, tag="patt", name="ps_t")
                        nc.tensor.transpose(
                            ps_t, oslabs[b][par][:, qb, fc * 128:(fc + 1) * 128], ident)
                        if b == 0:
                            nc.vector.tensor_copy(
                                otT_pars[b][:, fc, par, qb * 128:(qb + 1) * 128], ps_t)
                        else:
                            nc.scalar.copy(
                                otT_pars[b][:, fc, par, qb * 128:(qb + 1) * 128], ps_t)

        # ---- stage D: out-proj + residual ----
        x2Ts = [x2pool.tile([128, EC, S], BF16, tag="x2T", name=f"x2T{b}")
                for b in range(B)]
        for es in range(EC):
            for b in range(B):
                ps = pmain.tile([128, S], F32, tag="pmain")
                for c in range(EC):
                    nc.tensor.matmul(ps, lhsT=wout_sb[:, c, es * 128:(es + 1) * 128],
                                     rhs=otTs[b][:, c, :], start=(c == 0), stop=(c == EC - 1))
                if has_out_bias:
                    nc.vector.tensor_scalar(ps, ps, outb_sb[:, es:es + 1], None, ALU.add)
                nc.vector.tensor_tensor(x2Ts[b][:, es, :], ps,
                                        xts[b][:, es, HALO:T_EXT], ALU.add)

        # ---- stage E: LN2 ----
        ln2_rows = [ln_rows(b) for b in range(B)]
        for tt in range(2):
            for b in range(B):
                ln_chunk(x2Ts[b], ln2_rows[b], tt * 256, (tt + 1) * 256)
        x21s = []
        for b in range(B):
            x2T = x2Ts[b]
            rstd2_bf, z2row = ln2_rows[b][2][:, :S], ln2_rows[b][3][:, :S]
            x21 = x2pool.tile([128, EC, S], BF16, tag="x21", name=f"x21_{b}")
            for c in range(EC):
                eng = nc.vector if c < 2 else nc.gpsimd
                eng.tensor_tensor(x21[:, c, :], x2T[:, c, :], rstd2_bf, ALU.mult)
                eng.tensor_tensor(x21[:, c, :], x21[:, c, :], z2row, ALU.add)
            x21s.append(x21)

        # ---- stage F: MLP, batches interleaved ----
        h2Ts = [bigpool.tile([128, HC, S], BF16, tag="big", name=f"h2T{b}")
                for b in range(B)]
        for hs in range(HC):
            for b in range(B):
                ps = pmain.tile([128, S], F32, tag="pmain")
                for c in range(EC):
                    nc.tensor.matmul(ps, lhsT=w1_sb[:, c, hs * 128:(hs + 1) * 128],
                                     rhs=x21s[b][:, c, :], start=(c == 0), stop=(c == EC - 1))
                nc.scalar.activation(h2Ts[b][:, hs, :], ps, AF.Gelu,
                                     bias=vmlp_sb[:, hs:hs + 1])
        for es in range(EC):
            for b in range(B):
                ps = pmain.tile([128, S], F32, tag="pmain")
                for hc in range(HC):
                    nc.tensor.matmul(ps, lhsT=w2_sb[:, hc, es * 128:(es + 1) * 128],
                                     rhs=h2Ts[b][:, hc, :], start=(hc == 0), stop=(hc == HC - 1))
                if has_b2:
                    nc.vector.tensor_scalar(ps, ps, b2_sb[:, es:es + 1], None, ALU.add)
                yt = ypool.tile([128, S], F32, tag="yt", name="yt")
                nc.vector.tensor_tensor(yt, ps, x2Ts[b][:, es, :], ALU.add)
                nc.sync.dma_start(yT[b, es * 128:(es + 1) * 128, :], yt)

        for p in reversed(ctxstack):
            p.__exit__(None, None, None)

    return nc


_cached = {}


def _get_program(key):
    if key not in _cached:
        nc = build_program(*key)
        _legalize_waits(nc.m)
        _cached[key] = nc
    return _cached[key]


def _prepare_core_inputs(inputs):
    """Host-side folding + sharding. Returns (flags_key, in_maps list)."""
    x = np.asarray(inputs["x"], np.float32)
    ln1_g = np.asarray(inputs["ln1_g"], np.float32)
    ln1_b = np.asarray(inputs["ln1_b"], np.float32)
    qkv_w = np.asarray(inputs["qkv_w"], np.float32)
    qkv_b = np.asarray(inputs["qkv_b"], np.float32)
    out_w = np.asarray(inputs["out_w"], np.float32)
    out_b = np.asarray(inputs["out_b"], np.float32)
    ln2_g = np.asarray(inputs["ln2_g"], np.float32)
    ln2_b = np.asarray(inputs["ln2_b"], np.float32)
    w1 = np.asarray(inputs["w1"], np.float32)
    b1 = np.asarray(inputs["b1"], np.float32)
    w2 = np.asarray(inputs["w2"], np.float32)
    b2 = np.asarray(inputs["b2"], np.float32)

    # fold LN1 gain into qkv_w; fold attention 1/sqrt(D) into the Q part
    qscale = 1.0 / np.sqrt(D)
    wqkv_eff = ln1_g[:, None] * qkv_w
    vqkv = ln1_b @ qkv_w + qkv_b          # [3E]
    wqkv_eff[:, :E] *= qscale
    vqkv = vqkv.copy()
    vqkv[:E] *= qscale
    # fold LN2 gain into w1
    w1_eff = ln2_g[:, None] * w1
    vmlp = ln2_b @ w1 + b1                # [HID]

    has_qk_bias = bool(np.any(vqkv[: 2 * E] != 0.0))
    has_v_bias = bool(np.any(vqkv[2 * E:] != 0.0))
    has_out_bias = bool(np.any(out_b != 0.0))
    has_b2 = bool(np.any(b2 != 0.0))
    key = (has_qk_bias, has_v_bias, has_out_bias, has_b2)

    wqkv_bf = wqkv_eff.astype(ml_dtypes.bfloat16)
    wout_bf = out_w.astype(ml_dtypes.bfloat16)
    w1_bf = w1_eff.astype(ml_dtypes.bfloat16)
    w2_bf = w2.astype(ml_dtypes.bfloat16)

    # x transposed per batch with halo: [B, E, T_EXT], bf16 residual stream
    xT_full = np.ascontiguousarray(x.transpose(0, 2, 1))  # [B, E, L]
    in_maps = []
    for c in range(N_CORES):
        s = c * S
        xTe = np.zeros((B, E, T_EXT), np.float32)
        lo = s - HALO
        src_lo = max(lo, 0)
        xTe[:, :, src_lo - lo:] = xT_full[:, :, src_lo:s + S]
        vones = np.ones(KB * 128, np.float32)
        if c == 0:
            vones[:128] = 0.0
        im = {
            "xT": xTe.astype(ml_dtypes.bfloat16),
            "wqkv": wqkv_bf,
            "wout": wout_bf,
            "w1": w1_bf,
            "w2": w2_bf,
            "vones": vones.astype(ml_dtypes.bfloat16),
            "vmlp": vmlp.astype(np.float32),
        }
        if has_qk_bias:
            im["vqk"] = vqkv[: 2 * E].astype(np.float32)
        if has_v_bias:
            im["vvb"] = vqkv[2 * E:].astype(np.float32)
        if has_out_bias:
            im["outb"] = out_b.astype(np.float32)
        if has_b2:
            im["b2v"] = b2.astype(np.float32)
        in_maps.append(im)
    return key, in_maps


_last_results = None


def kernel(**inputs) -> np.ndarray:
    global _last_results
    key, in_maps = _prepare_core_inputs(inputs)
    nc = _get_program(key)
    res = run_bass_kernel_spmd(nc, in_maps, core_ids=list(range(N_CORES)))
    _last_results = res
    out = np.empty((B, L, E), np.float32)
    for c in range(N_CORES):
        yT = res.results[c]["yT"]          # [B, E, S]
        out[:, c * S:(c + 1) * S, :] = yT.transpose(0, 2, 1)
    return out


# revision 38
# speedup vs baseline: 1.0102x; 1.0050x over previous
"""Trainium2 Bass kernel for nn_DilatedAttention (B=2, L=4096, E=512, H=8, D=64,
dilation=2, window=256, causal, pre-norm transformer block with MLP).

Strategy
--------
* 8 cores, sequence-parallel: core c owns tokens [512c, 512c+512) of both
  batches, with a 256-token K/V halo before its range (zero-padded on core 0).
* Dilation-2 + causal + window couples only equal-parity positions; after
  de-interleaving by parity the mask is a causal sliding-window (window=128)
  attention over a length-2048 subsequence.  Parity de-interleave is a
  stride-2 slice on the free axis (feature-major layout).
* Both batches are interleaved instruction-by-instruction through every
  stage so each engine always has an independent stream to hide
  cross-engine dependency latency.
* Attention: per (head, parity) the scores for (query-block, key-slot) pairs
  land in one PSUM bank [128, 4, 128]; a single Exp per batch produces the
  probability tile, two strided affine_selects apply the sliding-window
  triangles for both batches at once, and the softmax denominator comes from
  a "valid key" ones-column appended to V.  Normalization is one batched
  reciprocal plus a per-qblock scale on DVE.
* bf16 residual stream: LayerNorm statistics matmuls read the residual
  directly (no fp32->bf16 shadow copies); squares computed on DVE.
* LN decomposition: h = x*rstd + z with z = -mu*rstd.  The per-token
  mean/var sums are computed with a ones-MATRIX stationary operand so the
  statistics land replicated across all 128 partitions: every subsequent
  chain op is a fast full-width [128,N] op (a [1,N] DVE op is ~6x slower),
  and x1 = x*rstd + z reads the slabs directly with no broadcast matmuls.
  The query-half token range is processed first so QKV starts while the
  halo half's statistics are still in flight.  LN gains fold into weights
  on the host.
"""

import os
import sys
import types
import numpy as np
import ml_dtypes

import concourse.bass as bass
import concourse.mybir as mybir
import concourse.tile as tile
from concourse.bass_utils import run_bass_kernel_spmd
from concourse.masks import make_identity


def _install_ntff_hook_shim():
    """This image's antenv lacks axon_hooks; bass_utils imports it when
    BASS_TRACE is set.  Provide the ctypes-based NTFF hook (or a None hook)
    so tracing works — and never crashes — in any environment."""
    try:
        import antenv
    except ImportError:
        return
    try:
        from antenv.axon_hooks import get_axon_ntff_profile_hook  # noqa: F401
        return  # real module present
    except ImportError:
        pass
    import ctypes
    import contextlib

    hook = None
    so_path = "/opt/axon/libaxon_pjrt.so"
    if os.path.exists(so_path):
        try:
            lib = ctypes.CDLL(so_path)
            if hasattr(lib, "axon_start_nrt_profile"):
                lib.axon_start_nrt_profile.argtypes = [
                    ctypes.POINTER(ctypes.c_int64), ctypes.c_size_t]
                lib.axon_start_nrt_profile.restype = ctypes.c_int64
                lib.axon_stop_nrt_profile.argtypes = [ctypes.c_char_p]
                lib.axon_stop_nrt_profile.restype = ctypes.c_int64

                @contextlib.contextmanager
                def _hook(output_dir, device_ids):
                    import jax
                    jax.devices()
                    if device_ids:
                        ids = (ctypes.c_int64 * len(device_ids))(*device_ids)
                        rc = lib.axon_start_nrt_profile(ids, len(device_ids))
                    else:
                        rc = lib.axon_start_nrt_profile(None, 0)
                    if rc != 0:
                        raise RuntimeError(f"axon_start_nrt_profile rc={rc}")
                    try:
                        yield
                    finally:
                        lib.axon_stop_nrt_profile(str(output_dir).encode())

                hook = _hook
        except OSError:
            hook = None

    mod = types.ModuleType("antenv.axon_hooks")
    mod.get_axon_ntff_profile_hook = lambda: hook
    mod.set_axon_ntff_profile_hook = lambda h: None
    sys.modules["antenv.axon_hooks"] = mod
    antenv.axon_hooks = mod


_install_ntff_hook_shim()

F32 = mybir.dt.float32
BF16 = mybir.dt.bfloat16
AF = mybir.ActivationFunctionType
ALU = mybir.AluOpType

# problem constants
B, L, E, H, D = 2, 4096, 512, 8, 64
HID = 2048
EPS = 1e-5
WIN, DIL = 256, 2
N_CORES = 8
S = L // N_CORES          # tokens per core per batch (512)
HALO = WIN                # kv halo tokens (256)
T_EXT = S + HALO          # 768
EC = E // 128             # 4 feature chunks
HC = HID // 128           # 16 hidden chunks
NQ = S // 2               # queries per parity (256)
KB = (NQ + 128) // 128    # key blocks per parity (3)
QB = NQ // 128            # query blocks per parity (2)


def _legalize_waits(m, max_waits=1):
    """The walrus build here accepts only one sync-wait command per lowered
    instruction; hoist extras onto same-engine NoOps placed just before."""
    for fn in m.functions:
        for blk in fn.blocks:
            new_list = []
            for ins in blk.instructions:
                si = ins.sync_info
                if si is not None and si.on_wait is not None and len(si.on_wait) > max_waits:
                    waits = list(si.on_wait)
                    extra, keep = waits[:-max_waits], waits[-max_waits:]
                    k = 0
                    while extra:
                        chunk, extra = extra[:max_waits], extra[max_waits:]
                        nop = mybir.InstNoOp(name=f"{ins.name}-wsplit{k}", ins=[], outs=[])
                        nop.engine = ins.engine
                        nop.sync_info = mybir.SyncInfo(on_wait=chunk, on_update=[])
                        new_list.append(nop)
                        k += 1
                    si.on_wait = keep
                new_list.append(ins)
            blk.instructions = new_list


def build_program(has_qk_bias: bool, has_v_bias: bool, has_out_bias: bool, has_b2: bool):
    nc = bass.Bass("TRN2", target_bir_lowering=False, debug=False)

    # ---- DRAM I/O ----
    xT = nc.dram_tensor("xT", [B, E, T_EXT], BF16, kind="ExternalInput").ap()
    wqkv = nc.dram_tensor("wqkv", [E, 3 * E], BF16, kind="ExternalInput").ap()
    wout = nc.dram_tensor("wout", [E, E], BF16, kind="ExternalInput").ap()
    w1 = nc.dram_tensor("w1", [E, HID], BF16, kind="ExternalInput").ap()
    w2 = nc.dram_tensor("w2", [HID, E], BF16, kind="ExternalInput").ap()
    vones_in = nc.dram_tensor("vones", [KB * 128], BF16, kind="ExternalInput").ap()
    vmlp_in = nc.dram_tensor("vmlp", [HID], F32, kind="ExternalInput").ap()
    if has_qk_bias:
        vqk_in = nc.dram_tensor("vqk", [2 * E], F32, kind="ExternalInput").ap()
    if has_v_bias:
        vvb_in = nc.dram_tensor("vvb", [E], F32, kind="ExternalInput").ap()
    if has_out_bias:
        outb_in = nc.dram_tensor("outb", [E], F32, kind="ExternalInput").ap()
    if has_b2:
        b2_in = nc.dram_tensor("b2v", [E], F32, kind="ExternalInput").ap()
    yT = nc.dram_tensor("yT", [B, E, S], F32, kind="ExternalOutput").ap()

    with tile.TileContext(nc) as tc:
        ctxstack = []

        def pool(name, bufs, space="SBUF"):
            p = tc.tile_pool(name=name, bufs=bufs, space=space)
            ctxstack.append(p)
            return p.__enter__()

        wpool = pool("wpool", 1)
        xpool = pool("xpool", 2)
        bigpool = pool("bigpool", 2)   # x1 (stages A-B), then h2T (stage F)
        qkpool = pool("qkpool", 2)
        vpool = pool("vpool", 1)
        ptpool = pool("ptpool", 6)
        ospool = pool("ospool", 1)
        otpool = pool("otpool", 2)
        x2pool = pool("x2pool", 2)
        ypool = pool("ypool", 3)
        stpool = pool("stpool", 2)
        sqpool = pool("sqpool", 2)
        rpool = pool("rpool", 4)

        pstat = pool("pstat", 2, space="PSUM")
        pmain = pool("pmain", 3, space="PSUM")
        patt = pool("patt", 3, space="PSUM")

        # ---- x for both batches first (critical path), then weights.
        # Spread the startup loads over three DGE queues so both batches'
        # x and the qkv weights land in parallel. ----
        xts = []
        for b in range(B):
            xt = xpool.tile([128, EC, T_EXT], BF16, tag="xt", name=f"xt{b}")
            q = nc.sync if b == 0 else nc.gpsimd
            q.dma_start(xt, xT[b].rearrange("(c p) t -> p c t", p=128))
            xts.append(xt)
        wqkv_sb = wpool.tile([128, EC, 3 * E], BF16)
        nc.scalar.dma_start(wqkv_sb, wqkv.rearrange("(c p) f -> p c f", p=128))
        # small constants on the sync queue
        vmlp_sb = wpool.tile([128, HC], F32)
        nc.sync.dma_start(vmlp_sb, vmlp_in.rearrange("(s p) -> p s", p=128))
        vones_sb = wpool.tile([128, KB], BF16)
        nc.sync.dma_start(vones_sb, vones_in.rearrange("(k p) -> p k", p=128))
        if has_qk_bias:
            vqk_sb = wpool.tile([128, 8], F32)
            nc.sync.dma_start(vqk_sb, vqk_in.rearrange("(s p) -> p s", p=128))
        if has_v_bias:
            vvb_sb = wpool.tile([128, E], F32)
            nc.sync.dma_start(vvb_sb, vvb_in[None, :].to_broadcast([128, E]))
        if has_out_bias:
            outb_sb = wpool.tile([128, EC], F32)
            nc.sync.dma_start(outb_sb, outb_in.rearrange("(s p) -> p s", p=128))
        if has_b2:
            b2_sb = wpool.tile([128, EC], F32)
            nc.sync.dma_start(b2_sb, b2_in.rearrange("(s p) -> p s", p=128))
        # later-needed weights on the second (Activation) HWDGE queue
        wout_sb = wpool.tile([128, EC, E], BF16)
        nc.scalar.dma_start(wout_sb, wout.rearrange("(c p) f -> p c f", p=128))
        w1_sb = wpool.tile([128, EC, HID], BF16)
        nc.scalar.dma_start(w1_sb, w1.rearrange("(c p) f -> p c f", p=128))
        w2_sb = wpool.tile([128, HC, E], BF16)
        nc.scalar.dma_start(w2_sb, w2.rearrange("(c p) f -> p c f", p=128))

        ident = wpool.tile([128, 128], BF16)
        make_identity(nc, ident)
        ones_mat = wpool.tile([128, 128], BF16)
        nc.vector.memset(ones_mat, 1.0)
        eps_t = wpool.tile([128, 1], F32)
        nc.vector.memset(eps_t, EPS)

        def ln_rows(b):
            """Per-batch LN statistic slab tiles, replicated across all 128
            partitions so every chain op runs full-width (a [1,N] DVE op is
            ~6x slower than a [128,N] one).  Reused for LN1 then LN2."""
            return (stpool.tile([128, T_EXT], F32, tag="mu", name=f"mu{b}"),
                    stpool.tile([128, T_EXT], F32, tag="sttmp", name=f"sttmp{b}"),
                    stpool.tile([128, T_EXT], BF16, tag="strbf", name=f"strbf{b}"),
                    stpool.tile([128, T_EXT], BF16, tag="stz", name=f"stz{b}"))

        def ln_chunk(xt_ap, rows, t0, t1):
            """Emit the LN statistic chain for token range [t0, t1): per-token
            mean/var over the feature (partition x chunk) axis — replicated to
            all partitions via a ones-matrix matmul — then rstd =
            1/sqrt(var+eps) (bf16) and z = -mu*rstd."""
            mu_full, tmp_full, rstd_bf, zrow = rows
            mu_neg, tmp = mu_full[:, t0:t1], tmp_full[:, t0:t1]
            ps_s_full = pstat.tile([128, 384], F32, tag="pstat", name="ps_s_full")
            ps_s = ps_s_full[:, : t1 - t0]
            for c in range(EC):
                nc.tensor.matmul(ps_s, lhsT=ones_mat, rhs=xt_ap[:, c, t0:t1],
                                 start=(c == 0), stop=(c == EC - 1))
            nc.scalar.mul(mu_neg, ps_s, -1.0 / E)
            ps_q_full = pstat.tile([128, 384], F32, tag="pstat", name="ps_q_full")
            ps_q = ps_q_full[:, : t1 - t0]
            for c in range(EC):
                xsq_full = sqpool.tile([128, 512], BF16, tag="xsq", name="xsq")
                xsq = xsq_full[:, : t1 - t0]
                nc.vector.tensor_tensor(xsq, xt_ap[:, c, t0:t1],
                                        xt_ap[:, c, t0:t1], ALU.mult)
                nc.tensor.matmul(ps_q, lhsT=ones_mat, rhs=xsq,
                                 start=(c == 0), stop=(c == EC - 1))
            nc.scalar.mul(tmp, ps_q, 1.0 / E)
            # var = E[x^2] - mu^2 ; rstd = 1/sqrt(var+eps) ; z = -mu*rstd
            musq_full = sqpool.tile([128, 384], F32, tag="musq", name="musq")
            musq = musq_full[:, : t1 - t0]
            nc.scalar.square(musq, mu_neg)
            nc.vector.tensor_tensor(tmp, tmp, musq, ALU.subtract)
            nc.scalar.activation(tmp, tmp, AF.Sqrt, bias=eps_t)
            with nc.allow_low_precision(reason="rstd is consumed in bf16 anyway"):
                nc.vector.reciprocal(rstd_bf[:, t0:t1], tmp)
            nc.vector.tensor_tensor(zrow[:, t0:t1], mu_neg, rstd_bf[:, t0:t1],
                                    ALU.mult)

        # ---- stage A: LN1 stats + x1 = x*rstd + z (both batches).
        # The query/second-half token range (tt=1) is computed first so
        # stage B can start while the halo half's statistics are still in
        # flight. ----
        x1s = [bigpool.tile([128, EC, T_EXT], BF16, tag="big", name=f"x1_{b}")
               for b in range(B)]
        ln1_rows = [ln_rows(b) for b in range(B)]
        for tt in (1, 0):
            t0, t1 = tt * 384, (tt + 1) * 384
            for b in range(B):
                ln_chunk(xts[b], ln1_rows[b], t0, t1)
            for b in range(B):
                rstd_bf, zrow = ln1_rows[b][2], ln1_rows[b][3]
                x1 = x1s[b]
                for c in range(EC):
                    eng = nc.vector if c < 2 else nc.gpsimd
                    eng.tensor_tensor(x1[:, c, t0:t1], xts[b][:, c, t0:t1],
                                      rstd_bf[:, t0:t1], ALU.mult)
                    eng.tensor_tensor(x1[:, c, t0:t1], x1[:, c, t0:t1],
                                      zrow[:, t0:t1], ALU.add)

        # ---- stage B: QKV, batches interleaved per feature slice ----
        qkTs = [qkpool.tile([128, 8, T_EXT], BF16, tag="qkT", name=f"qkT{b}")
                for b in range(B)]
        for fs in range(8):
            # queries only need the core's own tokens; keys need the halo.
            # tt=1 ranges first: their x1 is ready earliest.
            spans = [(HALO, T_EXT)] if fs < 4 else [(384, T_EXT), (0, 384)]
            for t0, t1 in spans:
                for b in range(B):
                    ps_full = pmain.tile([128, 512], F32, tag="pmain", name="ps_full")
                    ps = ps_full[:, : t1 - t0]
                    for c in range(EC):
                        nc.tensor.matmul(ps, lhsT=wqkv_sb[:, c, fs * 128:(fs + 1) * 128],
                                         rhs=x1s[b][:, c, t0:t1],
                                         start=(c == 0), stop=(c == EC - 1))
                    qkT = qkTs[b]
                    if has_qk_bias:
                        if fs >= 4:
                            nc.vector.tensor_scalar(qkT[:, fs, t0:t1], ps,
                                                    vqk_sb[:, fs:fs + 1], None, ALU.add)
                        else:
                            nc.scalar.activation(qkT[:, fs, t0:t1], ps, AF.Identity,
                                                 bias=vqk_sb[:, fs:fs + 1])
                    else:
                        nc.scalar.copy(qkT[:, fs, t0:t1], ps)

        # V token-major, parity-separated, with the valid-key column appended
        vplus = [[vpool.tile([128, KB, H, D + 1], BF16, tag=f"vplus{b}{p}",
                             name=f"vplus{b}{p}") for p in range(2)]
                 for b in range(B)]
        for b in range(B):
            for par in range(2):
                nc.gpsimd.tensor_copy(vplus[b][par][:, :, :, D],
                                      vones_sb[:, :, None].to_broadcast([128, KB, H]))
        x1_pars = [x1s[b].rearrange("p c (t two) -> p c two t", two=2) for b in range(B)]
        for par in range(2):
            for kb in reversed(range(KB)):
                for b in range(B):
                    ps = pmain.tile([128, E], F32, tag="pmain")
                    for c in range(EC):
                        nc.tensor.matmul(
                            ps, lhsT=x1_pars[b][:, c, par, kb * 128:(kb + 1) * 128],
                            rhs=wqkv_sb[:, c, 2 * E:3 * E],
                            start=(c == 0), stop=(c == EC - 1))
                    pv = ps.rearrange("p (h d) -> p h d", h=H)
                    if has_v_bias:
                        nc.vector.tensor_tensor(vplus[b][par][:, kb, :, 0:D], pv,
                                                vvb_sb.rearrange("p (h d) -> p h d", h=H),
                                                ALU.add)
                    else:
                        nc.scalar.copy(vplus[b][par][:, kb, :, 0:D], pv)

        # ---- stage C: attention ----
        # scores per (par, h): PSUM [128 keys, (qb,slot), 128 queries], where
        # slot 0 -> key block qb (keep k>=q), slot 1 -> key block qb+1
        # (keep q>=k).  One Exp per batch; masks span both batches via the
        # joint pt tile [128, B, 4, 128].
        qkT_pars = [qkTs[b].rearrange("p s (t two) -> p s two t", two=2)
                    for b in range(B)]
        oslabs = [[ospool.tile([128, QB, E], BF16, tag=f"oslab{b}{p}",
                               name=f"oslab{b}{p}") for p in range(2)]
                  for b in range(B)]
        for par in range(2):
            for h in range(H):
                rb = (h % 2) * 64
                sl = h // 2
                pt = ptpool.tile([128, B, 2 * QB, 128], BF16, tag="pt")
                for b in range(B):
                    ps_sc = pmain.tile([128, 2 * QB, 128], F32, tag="pmain",
                                       name="ps_sc")
                    for qb in range(QB):
                        for slot in range(2):
                            kb = qb + slot
                            nc.tensor.matmul(
                                ps_sc[:, 2 * qb + slot, :],
                                lhsT=qkT_pars[b][rb:rb + 64, 4 + sl, par,
                                                 kb * 128:(kb + 1) * 128],
                                rhs=qkT_pars[b][rb:rb + 64, sl, par,
                                                128 + qb * 128:256 + qb * 128],
                                start=True, stop=True)
                    nc.scalar.activation(pt[:, b], ps_sc, AF.Exp)
                # sliding-window masks, both batches at once:
                # lo slots: keep k - q >= 0 ; hi slots: keep q - k >= 0
                nc.gpsimd.affine_select(
                    out=pt[:, :, 0::2, :], in_=pt[:, :, 0::2, :],
                    compare_op=ALU.is_ge, fill=0.0,
                    base=0, channel_multiplier=1,
                    pattern=[[0, B], [0, QB], [-1, 128]])
                nc.gpsimd.affine_select(
                    out=pt[:, :, 1::2, :], in_=pt[:, :, 1::2, :],
                    compare_op=ALU.is_ge, fill=0.0,
                    base=0, channel_multiplier=-1,
                    pattern=[[0, B], [0, QB], [1, 128]])
                for b in range(B):
                    ps_o = patt.tile([128, QB, D + 1], F32, tag="patt")
                    for qb in range(QB):
                        for slot in range(2):
                            nc.tensor.matmul(ps_o[:, qb, :],
                                             lhsT=pt[:, b, 2 * qb + slot, :],
                                             rhs=vplus[b][par][:, qb + slot, h, :],
                                             start=(slot == 0), stop=(slot == 1))
                    rin = rpool.tile([128, QB], F32, tag="rin", name="rin")
                    nc.vector.reciprocal(rin, ps_o[:, :, D])
                    for qb in range(QB):
                        nc.vector.tensor_scalar(
                            oslabs[b][par][:, qb, h * D:(h + 1) * D],
                            ps_o[:, qb, 0:D], rin[:, qb:qb + 1], None, ALU.mult)

        # transpose O back to feature-major, re-interleaving parities
        otTs = [otpool.tile([128, EC, S], BF16, tag="otT", name=f"otT{b}")
                for b in range(B)]
        otT_pars = [otTs[b].rearrange("p c (t two) -> p c two t", two=2)
                    for b in range(B)]
        for par in range(2):
            for qb in range(QB):
                for fc in range(EC):
                    for b in range(B):
                        ps_t = patt.tile([128, 128], BF16, tag="patt", name="ps_t")
                        nc.tensor.transpose(
                            ps_t, oslabs[b][par][:, qb, fc * 128:(fc + 1) * 128], ident)
                        if b == 0:
                            nc.vector.tensor_copy(
                                otT_pars[b][:, fc, par, qb * 128:(qb + 1) * 128], ps_t)
                        else:
                            nc.scalar.copy(
                                otT_pars[b][:, fc, par, qb * 128:(qb + 1) * 128], ps_t)

        # ---- stage D: out-proj + residual ----
        x2Ts = [x2pool.tile([128, EC, S], BF16, tag="x2T", name=f"x2T{b}")
                for b in range(B)]
        for es in range(EC):
            for b in range(B):
                ps = pmain.tile([128, S], F32, tag="pmain")
                for c in range(EC):
                    nc.tensor.matmul(ps, lhsT=wout_sb[:, c, es * 128:(es + 1) * 128],
                                     rhs=otTs[b][:, c, :], start=(c == 0), stop=(c == EC - 1))
                if has_out_bias:
                    nc.vector.tensor_scalar(ps, ps, outb_sb[:, es:es + 1], None, ALU.add)
                nc.vector.tensor_tensor(x2Ts[b][:, es, :], ps,
                                        xts[b][:, es, HALO:T_EXT], ALU.add)

        # ---- stage E: LN2 ----
        ln2_rows = [ln_rows(b) for b in range(B)]
        for tt in range(2):
            for b in range(B):
                ln_chunk(x2Ts[b], ln2_rows[b], tt * 256, (tt + 1) * 256)
        x21s = []
        for b in range(B):
            x2T = x2Ts[b]
            rstd2_bf, z2row = ln2_rows[b][2][:, :S], ln2_rows[b][3][:, :S]
            x21 = x2pool.tile([128, EC, S], BF16, tag="x21", name=f"x21_{b}")
            for c in range(EC):
                eng = nc.vector if c < 2 else nc.gpsimd
                eng.tensor_tensor(x21[:, c, :], x2T[:, c, :], rstd2_bf, ALU.mult)
                eng.tensor_tensor(x21[:, c, :], x21[:, c, :], z2row, ALU.add)
            x21s.append(x21)

        # ---- stage F: MLP, batches interleaved ----
        h2Ts = [bigpool.tile([128, HC, S], BF16, tag="big", name=f"h2T{b}")
                for b in range(B)]
        for hs in range(HC):
            for b in range(B):
                ps = pmain.tile([128, S], F32, tag="pmain")
                for c in range(EC):
                    nc.tensor.matmul(ps, lhsT=w1_sb[:, c, hs * 128:(hs + 1) * 128],
                                     rhs=x21s[b][:, c, :], start=(c == 0), stop=(c == EC - 1))
                nc.scalar.activation(h2Ts[b][:, hs, :], ps, AF.Gelu,
                                     bias=vmlp_sb[:, hs:hs + 1])
        for es in range(EC):
            for b in range(B):
                ps = pmain.tile([128, S], F32, tag="pmain")
                for hc in range(HC):
                    nc.tensor.matmul(ps, lhsT=w2_sb[:, hc, es * 128:(es + 1) * 128],
                                     rhs=h2Ts[b][:, hc, :], start=(hc == 0), stop=(hc == HC - 1))
                if has_b2:
                    nc.vector.tensor_scalar(ps, ps, b2_sb[:, es:es + 1], None, ALU.add)
                yt = ypool.tile([128, S], F32, tag="yt", name="yt")
                nc.vector.tensor_tensor(yt, ps, x2Ts[b][:, es, :], ALU.add)
                outq = nc.sync if (es + b) % 2 == 0 else nc.gpsimd
                outq.dma_start(yT[b, es * 128:(es + 1) * 128, :], yt)

        for p in reversed(ctxstack):
            p.__exit__(None, None, None)

    return nc


_cached = {}


def _get_program(key):
    if key not in _cached:
        nc = build_program(*key)
        _legalize_waits(nc.m)
        _cached[key] = nc
    return _cached[key]


def _prepare_core_inputs(inputs):
    """Host-side folding + sharding. Returns (flags_key, in_maps list)."""
    x = np.asarray(inputs["x"], np.float32)
    ln1_g = np.asarray(inputs["ln1_g"], np.float32)
    ln1_b = np.asarray(inputs["ln1_b"], np.float32)
    qkv_w = np.asarray(inputs["qkv_w"], np.float32)
    qkv_b = np.asarray(inputs["qkv_b"], np.float32)
    out_w = np.asarray(inputs["out_w"], np.float32)
    out_b = np.asarray(inputs["out_b"], np.float32)
    ln2_g = np.asarray(inputs["ln2_g"], np.float32)
    ln2_b = np.asarray(inputs["ln2_b"], np.float32)
    w1 = np.asarray(inputs["w1"], np.float32)
    b1 = np.asarray(inputs["b1"], np.float32)
    w2 = np.asarray(inputs["w2"], np.float32)
    b2 = np.asarray(inputs["b2"], np.float32)

    # fold LN1 gain into qkv_w; fold attention 1/sqrt(D) into the Q part
    qscale = 1.0 / np.sqrt(D)
    wqkv_eff = ln1_g[:, None] * qkv_w
    vqkv = ln1_b @ qkv_w + qkv_b          # [3E]
    wqkv_eff[:, :E] *= qscale
    vqkv = vqkv.copy()
    vqkv[:E] *= qscale
    # fold LN2 gain into w1
    w1_eff = ln2_g[:, None] * w1
    vmlp = ln2_b @ w1 + b1                # [HID]

    has_qk_bias = bool(np.any(vqkv[: 2 * E] != 0.0))
    has_v_bias = bool(np.any(vqkv[2 * E:] != 0.0))
    has_out_bias = bool(np.any(out_b != 0.0))
    has_b2 = bool(np.any(b2 != 0.0))
    key = (has_qk_bias, has_v_bias, has_out_bias, has_b2)

    wqkv_bf = wqkv_eff.astype(ml_dtypes.bfloat16)
    wout_bf = out_w.astype(ml_dtypes.bfloat16)
    w1_bf = w1_eff.astype(ml_dtypes.bfloat16)
    w2_bf = w2.astype(ml_dtypes.bfloat16)

    # x transposed per batch with halo: [B, E, T_EXT], bf16 residual stream
    xT_full = np.ascontiguousarray(x.transpose(0, 2, 1))  # [B, E, L]
    in_maps = []
    for c in range(N_CORES):
        s = c * S
        xTe = np.zeros((B, E, T_EXT), np.float32)
        lo = s - HALO
        src_lo = max(lo, 0)
        xTe[:, :, src_lo - lo:] = xT_full[:, :, src_lo:s + S]
        vones = np.ones(KB * 128, np.float32)
        if c == 0:
            vones[:128] = 0.0
        im = {
            "xT": xTe.astype(ml_dtypes.bfloat16),
            "wqkv": wqkv_bf,
            "wout": wout_bf,
            "w1": w1_bf,
            "w2": w2_bf,
            "vones": vones.astype(ml_dtypes.bfloat16),
            "vmlp": vmlp.astype(np.float32),
        }
        if has_qk_bias:
            im["vqk"] = vqkv[: 2 * E].astype(np.float32)
        if has_v_bias:
            im["vvb"] = vqkv[2 * E:].astype(np.float32)
        if has_out_bias:
            im["outb"] = out_b.astype(np.float32)
        if has_b2:
            im["b2v"] = b2.astype(np.float32)
        in_maps.append(im)
    return key, in_maps


_last_results = None


def kernel(**inputs) -> np.ndarray:
    global _last_results
    key, in_maps = _prepare_core_inputs(inputs)
    nc = _get_program(key)
    res = run_bass_kernel_spmd(nc, in_maps, core_ids=list(range(N_CORES)))
    _last_results = res
    out = np.empty((B, L, E), np.float32)
    for c in range(N_CORES):
        yT = res.results[c]["yT"]          # [B, E, S]
        out[:, c * S:(c + 1) * S, :] = yT.transpose(0, 2, 1)
    return out
